# revision 30
# baseline (speedup 1.0000x reference)
"""Trainium2 Bass kernel for nn_MAB_44057774522768 (Set-Transformer MAB block).

Reference computation (per batch b, with B=8, Sq=Sk=1024, D=512, H=8 heads,
dh=64):
    Qp = Q @ Wq.T + bq                  [Sq, D]
    Kp = K @ Wk.T + bk                  [Sk, D]
    Vp = K @ Wv.T + bv                  [Sk, D]
    scores_h = Qp_h @ Kp_h.T / sqrt(D)  per head  [Sq, Sk]
    A = softmax(scores, axis=-1)
    ctx_h = A_h @ Vp_h
    O1 = Qp + ctx                       (residual on projected Q)
    out = O1 + relu(O1 @ Wo.T + bo)     (FFN residual)

Sharding: pure data-parallel, batch b -> core b (B == 8 == n_cores).

Device-side layout: "feature-major" — activations stored transposed
[feature, seq] so every matmul contracts over the partition axis with zero
on-chip transposes.  All matmul operands are bf16 (PSUM accumulates fp32);
the 2e-2 harness tolerance leaves ample room.

Head-pair concurrency: heads 2t (partitions 0-63) and 2t+1 (64-127) issue
their score matmuls back-to-back with disjoint PE row-groups
(tile_position (0,0) / (64,0)), so the two K=64 matmuls overlap in the
128x128 array and the score phase runs at ~full PE rate.  Each head of the
pair accumulates context into its own PSUM tile (pc_A / pc_B).

Softmax: no max-subtraction needed (|scores/sqrt(512)| ~ 1).  Per 128-key
tile the exp runs either on the Scalar engine (exact table exp, bf16 out)
or on the Vector engine via a Schraudolph bit-trick
(int16(x*128/ln2 + 16248.6) reinterpreted as bf16 ~= exp(x), ~2% element
error, which washes out through the softmax), splitting the exp load across
both engines.  The softmax denominator rides as a ones-column appended to V
(row 64 of the ctx PSUM accumulator).

bv is NOT added to Vp on-device: softmax weights sum to 1, so A @ (Vp + bv)
== A @ Vp + bv; bv is added at the end and folded into the FFN bias
(bo2 = bo + Wo @ bv) on the host.

PSUM budget (8 banks): score tiles (tag "ps", 2 x [128,1024]f32 = 4 banks)
+ ctx accumulators (tag "pc", 2 x [65,1024]f32 = 4 banks).  Projection and
FFN matmuls borrow the "ps" ring, so Q/K tiles 1-3 drip into the attend
loop as fillers using PE slack.
"""

import math
import os

import numpy as np

import concourse.bass as bass
import concourse.mybir as mybir
import concourse.tile as tile
from concourse import bacc
from concourse.bass_utils import run_bass_kernel_spmd

B, SQ, SK, D = 8, 1024, 1024, 512
H, DH = 8, 64
N_CORES = 8
KC = D // 128  # 4 contraction chunks of 128 (din)
MT = D // 128  # 4 output-feature tiles of 128 (dout)
NQ = SQ // 512  # 2 moving chunks of 512 (seq)
KT8 = SK // 128  # 8 key-seq tiles of 128

F32 = mybir.dt.float32
BF16 = mybir.dt.bfloat16
I16 = mybir.dt.int16
I8 = mybir.dt.int8
F8E5 = mybir.dt.float8e5
ALU = mybir.AluOpType
ACTF = mybir.ActivationFunctionType
DROW = mybir.MatmulPerfMode.DoubleRow

_NC = None


def _build():
    nc = bacc.Bacc(None, target_bir_lowering=False, debug=False)

    dQT = nc.dram_tensor("QT", [D, SQ], BF16, kind="ExternalInput")
    dKT = nc.dram_tensor("KT", [D, SK], BF16, kind="ExternalInput")
    dWq = nc.dram_tensor("WqT", [D, D], BF16, kind="ExternalInput")  # [din,dout]
    dWk = nc.dram_tensor("WkT", [D, D], BF16, kind="ExternalInput")
    dWv = nc.dram_tensor("WvT", [D, D], BF16, kind="ExternalInput")
    dWo = nc.dram_tensor("WoT", [D, D], BF16, kind="ExternalInput")
    dBQ = nc.dram_tensor("BQ", [128, MT], F32, kind="ExternalInput")
    dBK = nc.dram_tensor("BK", [128, MT], F32, kind="ExternalInput")
    dBO2 = nc.dram_tensor("BO2", [128, MT], F32, kind="ExternalInput")
    dBV = nc.dram_tensor("BV", [128, MT], F32, kind="ExternalInput")
    dOT = nc.dram_tensor("OT", [D, SQ], F32, kind="ExternalOutput")

    dbg = os.environ.get("KDEBUG", "0") == "1"
    if dbg:
        dDQP = nc.dram_tensor("DQP", [128, MT, SQ], BF16, kind="ExternalOutput")
        dDKP = nc.dram_tensor("DKP", [128, MT, SK], BF16, kind="ExternalOutput")
        dDVPA = nc.dram_tensor("DVPA", [128, KT8 // 2, 2, H, 80], F8E5, kind="ExternalOutput")
        dDEXA = nc.dram_tensor("DEXA", [128, 2, SQ], F8E5, kind="ExternalOutput")
        dDEXB = nc.dram_tensor("DEXB", [128, 2, SQ], F8E5, kind="ExternalOutput")
        dDRB = nc.dram_tensor("DRB", [128, SQ], F32, kind="ExternalOutput")
        dDCB = nc.dram_tensor("DCB", [128, SQ], F32, kind="ExternalOutput")
        dDCN = nc.dram_tensor("DCN", [128, SQ], BF16, kind="ExternalOutput")
        dDO1 = nc.dram_tensor("DO1", [128, MT, SQ], BF16, kind="ExternalOutput")

    scale = 1.0 / math.sqrt(float(D))
    # Schraudolph exp in fp8e5 bit-space: int8(x*scale * 2^2/ln2 + (15*4 -
    # 0.0579*4)) reinterpreted as e5m2.  Scores*scale are ~N(0, 0.12), so the
    # int8 value stays in [50, 70] — no overflow/sign risk.
    sch_mul = (2.0**2 / math.log(2.0)) * scale
    sch_add = 15.0 * 4.0 - 0.0579 * 4.0

    with tile.TileContext(nc) as tc:
        with (
            tc.tile_pool(name="persist", bufs=1) as persist,
            tc.tile_pool(name="spool", bufs=2, space="PSUM") as spool,
            tc.tile_pool(name="cpool", bufs=2, space="PSUM") as cpool,
            tc.tile_pool(name="epool", bufs=6) as epool,
            tc.tile_pool(name="rpool", bufs=2) as rpool,
            tc.tile_pool(name="pairpool", bufs=2) as pairpool,
            tc.tile_pool(name="outpool", bufs=2) as outpool,
            tc.tile_pool(name="dpool", bufs=2, space="DRAM") as dpool,
        ):
            # ---- persistent SBUF tensors ----
            qt = persist.tile([128, KC, SQ], BF16)
            kt = persist.tile([128, KC, SK], BF16)
            wq = persist.tile([128, KC, D], BF16)
            wk = persist.tile([128, KC, D], BF16)
            wv = persist.tile([128, KC, D], BF16)
            wo = persist.tile([128, KC, D], BF16)
            bq = persist.tile([128, MT], F32)
            bk = persist.tile([128, MT], F32)
            bo2 = persist.tile([128, MT], F32)
            bv = persist.tile([128, MT], F32)
            qpb = persist.tile([128, MT, SQ], BF16)
            kpb = persist.tile([128, MT, SK], BF16)
            # Vp for DoubleRow ctx: [k, key-tile-pair mp, j in pair, head,
            # dh+ones column], padded to 80 so the lhsT pair-stride is a
            # multiple of 16 bytes (DoubleRow LDWEIGHTS constraint).
            vpa = persist.tile([128, KT8 // 2, 2, H, 80], F8E5)
            o1 = persist.tile([128, MT, SQ], BF16)

            # ---- input DMAs, ordered by first use, split across queues ----
            nc.sync.dma_start(out=bq, in_=dBQ[:, :])
            nc.sync.dma_start(out=bk, in_=dBK[:, :])
            nc.sync.dma_start(out=bo2, in_=dBO2[:, :])
            nc.sync.dma_start(out=bv, in_=dBV[:, :])
            for kc in range(KC):
                nc.sync.dma_start(out=wq[:, kc, :], in_=dWq[kc * 128:(kc + 1) * 128, :])
                nc.scalar.dma_start(out=qt[:, kc, :], in_=dQT[kc * 128:(kc + 1) * 128, :])
            for kc in range(KC):
                nc.sync.dma_start(out=wk[:, kc, :], in_=dWk[kc * 128:(kc + 1) * 128, :])
                keng = nc.scalar if kc % 2 == 0 else nc.gpsimd
                keng.dma_start(out=kt[:, kc, :], in_=dKT[kc * 128:(kc + 1) * 128, :])
            for kc in range(KC):
                nc.gpsimd.dma_start(out=wv[:, kc, :], in_=dWv[kc * 128:(kc + 1) * 128, :])
            for kc in range(KC):
                nc.gpsimd.dma_start(out=wo[:, kc, :], in_=dWo[kc * 128:(kc + 1) * 128, :])

            # ones column for the fused softmax denominator
            nc.vector.memset(vpa[:, :, :, :, DH:DH + 1], 1.0)

            def project(dst, w, rhs_src, bias_ap, m):
                """dst[:, m, :] = (w[:,:,m-tile].T @ rhs_src) + bias, both
                512-chunks through one [128,1024] PSUM tile.  Bias-add on ACT
                (Identity with per-partition bias) to keep DVE free for the
                softmax pipeline."""
                pp = spool.tile([128, SQ], F32, name="pp", tag="ps")
                for n in range(NQ):
                    nsl = slice(n * 512, (n + 1) * 512)
                    for kc in range(KC):
                        nc.tensor.matmul(
                            pp[:, nsl],
                            w[:, kc, m * 128:(m + 1) * 128],
                            rhs_src[:, kc, nsl],
                            start=(kc == 0),
                            stop=(kc == KC - 1),
                        )
                nc.scalar.activation(dst[:, m, :], pp[:, :], ACTF.Identity, bias=bias_ap)

            def project_v2(mtp):
                """vpa[:, mtp, :, :, 0:64] = Vp for key tiles 2mtp, 2mtp+1."""
                pv = spool.tile([128, SQ], F32, name="pv", tag="ps")
                for j in range(2):
                    mt = 2 * mtp + j
                    for kc in range(KC):
                        nc.tensor.matmul(
                            pv[:, j * 512:(j + 1) * 512],
                            kt[:, kc, mt * 128:(mt + 1) * 128],
                            wv[:, kc, :],
                            start=(kc == 0),
                            stop=(kc == KC - 1),
                        )
                nc.scalar.activation(
                    vpa[:, mtp, 0:2, :, 0:DH],
                    pv[:, :].rearrange("p (m h d) -> p m h d", m=2, h=H),
                    ACTF.Copy,
                )

            # deferred work, drip-fed into the attend loop's PE slack
            fillers = []
            for mtp in range(1, 4):
                fillers.append(lambda mtp=mtp: project_v2(mtp))
            for t in range(1, 4):
                fillers.append(lambda t=t: project(qpb, wq, qt, bq[:, t:t + 1], t))
                fillers.append(lambda t=t: project(kpb, wk, kt, bk[:, t:t + 1], t))

            def attend_pair(t, fill_at):
                """Heads 2t (partitions 0-63) and 2t+1 (64-127).  Head A's
                exp runs on ACT, head B's on DVE (Schraudolph), concurrently.
                ctx(m-1) is emitted AFTER scores(m) so it executes inside the
                exp(m) wait window instead of on the scores->exp->scores
                critical chain (the 2-deep PSUM ring forces scores(m+1) to
                wait for exp(m))."""
                pca = cpool.tile([DH + 1, SQ], F32, name="pca", tag="pc")
                pcb = cpool.tile([DH + 1, SQ], F32, name="pcb", tag="pc")

                def emit_ctx(mp, exa_p, exb_p):
                    for pc, exp_p, h in ((pca, exa_p, 2 * t), (pcb, exb_p, 2 * t + 1)):
                        for n in range(NQ):
                            nsl = slice(n * 512, (n + 1) * 512)
                            nc.tensor.matmul(
                                pc[:, nsl],
                                vpa[:, mp, :, h, 0:DH + 1],
                                exp_p[:, :, nsl],
                                start=(mp == 0),
                                stop=(mp == KT8 // 2 - 1),
                                perf_mode=DROW,
                            )

                pending = None
                exa_p = exb_p = None
                for m in range(KT8):
                    mp, j = divmod(m, 2)
                    msl = slice(m * 128, (m + 1) * 128)
                    psa = spool.tile([128, SQ], F32, name="psa", tag="ps")
                    psb = spool.tile([128, SQ], F32, name="psb", tag="ps")
                    for n in range(NQ):
                        nsl = slice(n * 512, (n + 1) * 512)
                        nc.tensor.matmul(
                            psa[:, nsl], kpb[0:64, t, msl], qpb[0:64, t, nsl],
                            start=True, stop=True,
                        )
                        nc.tensor.matmul(
                            psb[:, nsl], kpb[64:128, t, msl], qpb[64:128, t, nsl],
                            start=True, stop=True,
                        )
                    if j == 0:
                        exa_p = epool.tile([128, 2, SQ], F8E5, name="exa", tag="ex")
                        exb_p = epool.tile([128, 2, SQ], F8E5, name="exb", tag="ex")
                    nc.scalar.activation(exa_p[:, j, :], psa[:, :], ACTF.Exp, scale=scale)
                    nc.vector.tensor_scalar(
                        exb_p.bitcast(I8)[:, j, :], psb[:, :],
                        sch_mul, sch_add, ALU.mult, ALU.add,
                    )
                    if dbg and t == 0 and m == 1:
                        nc.sync.dma_start(out=dDEXA[:, :, :], in_=exa_p[:, :, :])
                        nc.sync.dma_start(out=dDEXB[:, :, :], in_=exb_p[:, :, :])
                    if pending is not None:
                        emit_ctx(*pending)
                        pending = None
                    if j == 1:
                        pending = (mp, exa_p, exb_p)
                    if m in fill_at and fillers:
                        fillers.pop(0)()
                emit_ctx(*pending)
                # normalization tail: reciprocal of the denominator row
                # (partition 64 of each pc), partition-broadcast via a DRAM
                # bounce, then o1 = qpb + ctx * (1/den).  Head A's ctx rows
                # are already at partitions 0-63 and multiply straight out of
                # PSUM; head B's rows must cross to partitions 64-127, which
                # only a DMA can do, so they bounce PSUM -> SBUF (ACT copy)
                # -> DMA partition-move -> DVE mul.
                rb = pairpool.tile([128, SQ], F32, name="rb", tag="rb")
                # head B: ctx rows + den row to SBUF in one ACT copy; head A:
                # only the den row needs SBUF (ctx multiplies out of PSUM).
                cb = pairpool.tile([128, SQ], F32, name="cb", tag="cb")
                nc.scalar.activation(cb[0:DH + 1, :], pcb[0:DH + 1, :], ACTF.Copy)
                da = rpool.tile([128, SQ], F32, name="da", tag="da")
                nc.scalar.activation(da[0:DH + 1, :], pca[0:DH + 1, :], ACTF.Copy)
                for hh, den in ((0, da), (1, cb)):
                    hb = 64 * hh
                    scr = rpool.tile([128, SQ], F32, name="scr", tag="scr")
                    nc.vector.reciprocal_approx_fast(scr[0:DH + 1, :], den[0:DH + 1, :])
                    rec_d = dpool.tile([1, SQ], F32, name="rec_d", tag="rec_d")
                    nc.sync.dma_start(out=rec_d[:, :], in_=scr[DH:DH + 1, :])
                    bsrc = bass.AP(
                        tensor=rec_d[0:1, :].tensor,
                        offset=rec_d[0:1, :].offset,
                        ap=[[0, 64], [1, SQ]],
                    )
                    nc.sync.dma_start(out=rb[hb:hb + 64, :], in_=bsrc)
                nc.gpsimd.dma_start(out=cb[64:128, :], in_=cb[0:64, :])
                cn = pairpool.tile([128, SQ], BF16, name="cn", tag="cn")
                nc.vector.tensor_mul(cn[0:64, :], pca[0:DH, :], rb[0:64, :])
                nc.vector.tensor_mul(cn[64:128, :], cb[64:128, :], rb[64:128, :])
                nc.vector.tensor_add(o1[:, t, :], cn[:, :], qpb[:, t, :])
                if dbg and t == 0:
                    nc.sync.dma_start(out=dDRB[:, :], in_=rb[:, :])
                    nc.sync.dma_start(out=dDCB[:, :], in_=cb[:, :])
                    nc.sync.dma_start(out=dDCN[:, :], in_=cn[:, :])

            # ---- FFN: out = O1 + bv + relu(WoT.T @ O1 + bo2) ----
            # Split per m-tile into a kc0-2 pass (only needs o1 tiles 0-2,
            # ready before the last attend pair's tail) and a kc3 pass.  The
            # kc0-2 passes ride pair 3's filler slots so the PE chews on them
            # while pair 3's softmax/normalization runs instead of idling.
            def ffn_p1(m):
                pf = spool.tile([128, SQ], F32, name="pf", tag="ps")
                for n in range(NQ):
                    nsl = slice(n * 512, (n + 1) * 512)
                    for kc in range(KC - 1):
                        nc.tensor.matmul(
                            pf[:, nsl],
                            wo[:, kc, m * 128:(m + 1) * 128],
                            o1[:, kc, nsl],
                            start=(kc == 0),
                            stop=False,
                        )
                return pf

            def ffn_p2(m, pf):
                for n in range(NQ):
                    nsl = slice(n * 512, (n + 1) * 512)
                    nc.tensor.matmul(
                        pf[:, nsl],
                        wo[:, KC - 1, m * 128:(m + 1) * 128],
                        o1[:, KC - 1, nsl],
                        start=False,
                        stop=True,
                    )
                rf = outpool.tile([128, SQ], F32, name="rf", tag="rf")
                nc.scalar.activation(rf[:, :], pf[:, :], ACTF.Relu, bias=bo2[:, m:m + 1])
                ot = outpool.tile([128, SQ], F32, name="ot", tag="ot")
                nc.vector.scalar_tensor_tensor(
                    ot[:, :], rf[:, :], bv[:, m:m + 1], o1[:, m, :],
                    ALU.add, ALU.add,
                )
                eng = nc.gpsimd if m % 2 == 0 else nc.sync
                eng.dma_start(out=dOT[m * 128:(m + 1) * 128, :], in_=ot[:, :])

            # ---- emission ----
            project(qpb, wq, qt, bq[:, 0:1], 0)
            project(kpb, wk, kt, bk[:, 0:1], 0)
            project_v2(0)
            attend_pair(0, fill_at={0, 1, 2, 4, 6})
            attend_pair(1, fill_at={0, 2, 4, 6})
            attend_pair(2, fill_at={0, 2, 4, 6})
            while fillers:
                fillers.pop(0)()
            attend_pair(3, fill_at=set())
            # FFN kc0-2 passes for the first two m-tiles run on the PE while
            # pair 3's normalization tail (ACT/DVE/DMA) completes.
            pf0 = ffn_p1(0)
            pf1 = ffn_p1(1)

            if dbg:
                nc.sync.dma_start(out=dDQP[:, :, :], in_=qpb[:, :, :])
                nc.sync.dma_start(out=dDKP[:, :, :], in_=kpb[:, :, :])
                nc.sync.dma_start(out=dDVPA[:, :, :, :, :], in_=vpa[:, :, :, :, :])
                nc.sync.dma_start(out=dDO1[:, :, :], in_=o1[:, :, :])

            ffn_p2(0, pf0)
            pf2 = ffn_p1(2)
            ffn_p2(1, pf1)
            pf3 = ffn_p1(3)
            ffn_p2(2, pf2)
            ffn_p2(3, pf3)

    nc.compile()
    return nc


def _get_nc():
    global _NC
    if _NC is None:
        _NC = _build()
    return _NC


def _prep_inputs(Q, K, Wq, bq, Wk, bk, Wv, bv, Wo, bo):
    Q = np.asarray(Q, dtype=np.float32)
    K = np.asarray(K, dtype=np.float32)
    Wq = np.asarray(Wq, dtype=np.float32)
    Wk = np.asarray(Wk, dtype=np.float32)
    Wv = np.asarray(Wv, dtype=np.float32)
    Wo = np.asarray(Wo, dtype=np.float32)
    bq = np.asarray(bq, dtype=np.float32)
    bk = np.asarray(bk, dtype=np.float32)
    bv = np.asarray(bv, dtype=np.float32)
    bo = np.asarray(bo, dtype=np.float32)

    bo2 = (bo + Wo @ bv).astype(np.float32)

    def btile(b):
        return np.ascontiguousarray(b.reshape(MT, 128).T)

    import ml_dtypes
    bf = ml_dtypes.bfloat16
    shared = {
        "WqT": np.ascontiguousarray(Wq.T).astype(bf),
        "WkT": np.ascontiguousarray(Wk.T).astype(bf),
        "WvT": np.ascontiguousarray(Wv.T).astype(bf),
        "WoT": np.ascontiguousarray(Wo.T).astype(bf),
        "BQ": btile(bq),
        "BK": btile(bk),
        "BO2": btile(bo2),
        "BV": btile(bv),
    }
    in_maps = []
    for c in range(N_CORES):
        m = dict(shared)
        m["QT"] = np.ascontiguousarray(Q[c].T).astype(bf)
        m["KT"] = np.ascontiguousarray(K[c].T).astype(bf)
        in_maps.append(m)
    return in_maps


def run(inputs, trace=False):
    """Run on hardware; returns (output [B,SQ,D] f32, BassKernelResults)."""
    in_maps = _prep_inputs(
        inputs["Q"], inputs["K"], inputs["Wq"], inputs["bq"], inputs["Wk"],
        inputs["bk"], inputs["Wv"], inputs["bv"], inputs["Wo"], inputs["bo"],
    )
    nc = _get_nc()
    res = run_bass_kernel_spmd(
        nc, in_maps, core_ids=list(range(N_CORES)), trace=trace
    )
    out = np.stack(
        [res.results[c]["OT"].T for c in range(N_CORES)], axis=0
    ).astype(np.float32)
    return out, res


def kernel(**inputs):
    nh = inputs.get("num_heads", H)
    assert int(nh) == H, f"kernel hardcodes num_heads={H}, got {nh}"
    out, _ = run(inputs, trace=False)
    return out


if __name__ == "__main__":
    rng = np.random.default_rng(0)
    inputs = {
        "Q": rng.standard_normal((B, SQ, D), dtype=np.float32),
        "K": rng.standard_normal((B, SK, D), dtype=np.float32),
        "Wq": rng.standard_normal((D, D), dtype=np.float32) * 0.04,
        "bq": rng.standard_normal((D,), dtype=np.float32) * 0.04,
        "Wk": rng.standard_normal((D, D), dtype=np.float32) * 0.04,
        "bk": rng.standard_normal((D,), dtype=np.float32) * 0.04,
        "Wv": rng.standard_normal((D, D), dtype=np.float32) * 0.04,
        "bv": rng.standard_normal((D,), dtype=np.float32) * 0.04,
        "Wo": rng.standard_normal((D, D), dtype=np.float32) * 0.04,
        "bo": rng.standard_normal((D,), dtype=np.float32) * 0.04,
        "num_heads": H,
    }
    out = kernel(**inputs)
    print("out", out.shape, out.dtype, float(np.abs(out).max()))
